# revision 1
# baseline (speedup 1.0000x reference)
"""2x2 average pool + per-channel affine on 8 TRN2 NeuronCores.

Problem: x (16, 64, 512, 512) f32 -> out (16, 64, 256, 256) f32
  out[b,c,i,j] = weight[c] * mean(x[b,c,2i:2i+2,2j:2j+2]) + bias[c]

Sharding: pure data parallel over batch. Core k gets batches [2k, 2k+1]
(128 images of 512x512 per core), weight/bias replicated.

Per-core layout: partition p = (b_local*64 + c) -> one full image per
partition. Each iteration DMAs 16 input rows per partition (32 KiB
contiguous, 4 MiB per dma_start), does the vertical pool with one
tensor_tensor add (row pairs are adjacent in the free dim), the
horizontal pool with a stride-2 tensor_tensor add, and the per-channel
affine on the scalar engine (scale/bias are per-partition [128,1]
scalars since partition == (b_local, c)).
"""

import numpy as np

import concourse.bacc as bacc
import concourse.bass as bass
import concourse.mybir as mybir
import concourse.tile as tile
from concourse.bass_utils import run_bass_kernel_spmd

N_CORES = 8
B, C, S = 16, 64, 512
B_LOC = B // N_CORES            # 2 batches per core
P = B_LOC * C                   # 128 partitions = one image per partition
IMG = S * S                     # 262144 input elems per image
OS = S // 2                     # 256
OUT_IMG = OS * OS               # 65536 output elems per image
ROWS_PER_ITER = 16              # input rows loaded per iteration
CHUNK = ROWS_PER_ITER * S       # 8192 elems per partition per load (32 KiB)
N_ITERS = IMG // CHUNK          # 32
OUT_CHUNK = CHUNK // 4          # 2048 elems per partition per store

FP32 = mybir.dt.float32

_nc_cache = None


def _build(reps=1, rows=8, ibufs=6, vbufs=3, hbufs=3, obufs=4,
           store_eng="scalar", split_load=1, loop_n=0,
           inplace_v=False, inplace_y=False):
    # Bacc (not raw Bass): its finalize pass splits multi-sem waits into
    # event-semaphore instructions — TRN2 allows at most 1 wait per inst.
    # reps>1 repeats the full pass back-to-back in one NEFF (delta-timing).
    nc = bacc.Bacc("TRN2", target_bir_lowering=False, debug=False,
                   num_devices=N_CORES)
    chunk = rows * S             # input elems per partition per iteration
    n_iters = IMG // chunk
    out_chunk = chunk // 4

    x = nc.declare_dram_parameter("x", [P, IMG], FP32, isOutput=False)
    # affine[:, 0] = weight[c] / 4 (pool norm folded in), affine[:, 1] = bias[c]
    # (host-precomputed, already broadcast to the 128 partition images)
    affine = nc.declare_dram_parameter("affine", [P, 2], FP32, isOutput=False)
    out = nc.declare_dram_parameter("out", [P, OUT_IMG], FP32, isOutput=True)

    store = {"sync": nc.sync, "scalar": nc.scalar, "gpsimd": nc.gpsimd}[store_eng]

    with tile.TileContext(nc) as tc:
        with tc.tile_pool(name="consts", bufs=1) as cpool, \
             tc.tile_pool(name="ld", bufs=ibufs) as ipool, \
             tc.tile_pool(name="vmid", bufs=vbufs) as vpool, \
             tc.tile_pool(name="hmid", bufs=hbufs) as hpool, \
             tc.tile_pool(name="st", bufs=obufs) as opool:

            cb = cpool.tile([P, 2], FP32)
            nc.sync.dma_start(out=cb[:], in_=affine[:, :])
            s_ap = cb[:, 0:1]
            b_ap = cb[:, 1:2]

            import contextlib
            loop_ctx = tc.For_i(0, loop_n, 1) if loop_n else \
                contextlib.nullcontext()
            with loop_ctx:
              for i in range(n_iters * reps):
                i = i % n_iters
                t = ipool.tile([P, chunk], FP32)
                if split_load == 1:
                    nc.sync.dma_start(out=t[:],
                                      in_=x[:, i * chunk:(i + 1) * chunk])
                else:
                    part = chunk // split_load
                    for s_ in range(split_load):
                        nc.sync.dma_start(
                            out=t[:, s_ * part:(s_ + 1) * part],
                            in_=x[:, i * chunk + s_ * part:
                                  i * chunk + (s_ + 1) * part])

                # vertical pool: rows 2r and 2r+1 sit at free-dim offsets
                # (2r*S, (2r+1)*S) -> contiguous-stride add. Writing the
                # result into the front of t is safe: the DVE streams
                # monotonically and every write index trails its reads.
                tv = t[:].rearrange("p (r two w) -> p r two w", two=2, w=S)
                v_ap = t[:, 0:chunk // 2] if inplace_v else \
                    vpool.tile([P, chunk // 2], FP32, name="v", tag="v")[:]
                vv = v_ap.rearrange("p (r w) -> p r w", w=S)
                nc.vector.tensor_add(vv, tv[:, :, 0, :], tv[:, :, 1, :])

                # horizontal pool: adjacent column pairs, stride-2 operands
                vh = v_ap.rearrange("p (r j two) -> p r j two", two=2, j=OS)
                y = opool.tile([P, out_chunk], FP32)
                h_ap = y[:] if inplace_y else \
                    hpool.tile([P, out_chunk], FP32, name="h", tag="h")[:]
                hh = h_ap.rearrange("p (r j) -> p r j", j=OS)
                nc.vector.tensor_add(hh, vh[:, :, :, 0], vh[:, :, :, 1])

                # per-channel affine on the scalar engine:
                # y = Identity(h * (w[c]/4) + bias[c])
                nc.scalar.activation(y[:], h_ap,
                                     mybir.ActivationFunctionType.Identity,
                                     bias=b_ap, scale=s_ap)

                store.dma_start(out=out[:, i * out_chunk:(i + 1) * out_chunk],
                                in_=y[:])

    # run Bacc's legalization passes (multi-wait splitting, reg alloc, ...);
    # run_bass_via_pjrt serializes nc.m as-is and never finalizes.
    nc.finalize()
    return nc


def _get_nc():
    global _nc_cache
    if _nc_cache is None:
        _nc_cache = _build()
    return _nc_cache


def _make_in_maps(x, weight, bias):
    x = np.ascontiguousarray(np.asarray(x, dtype=np.float32))
    weight = np.asarray(weight, dtype=np.float32).reshape(C)
    bias = np.asarray(bias, dtype=np.float32).reshape(C)
    affine = np.stack([np.tile(weight * 0.25, B_LOC),
                       np.tile(bias, B_LOC)], axis=1)
    affine = np.ascontiguousarray(affine, dtype=np.float32)  # [P, 2]
    in_maps = []
    for k in range(N_CORES):
        shard = np.ascontiguousarray(
            x[k * B_LOC:(k + 1) * B_LOC].reshape(P, IMG))
        in_maps.append({"x": shard, "affine": affine})
    return in_maps


def run_sharded(x, weight, bias, trace=False, build_kw=None, **kw):
    """Run the SPMD kernel; returns (full_output, BassKernelResults)."""
    nc = _build(**build_kw) if build_kw else _get_nc()
    res = run_bass_kernel_spmd(nc, _make_in_maps(x, weight, bias),
                               core_ids=list(range(N_CORES)), trace=trace, **kw)
    outs = [res.results[k]["out"].reshape(B_LOC, C, OS, OS)
            for k in range(N_CORES)]
    return np.concatenate(outs, axis=0), res


def kernel(x, weight, bias):
    out, _ = run_sharded(x, weight, bias, trace=False)
    return out



# revision 12
# speedup vs baseline: 1.3906x; 1.3906x over previous
"""2x2 average pool + per-channel affine on 8 TRN2 NeuronCores.

Problem: x (16, 64, 512, 512) f32 -> out (16, 64, 256, 256) f32
  out[b,c,i,j] = weight[c] * mean(x[b,c,2i:2i+2,2j:2j+2]) + bias[c]

Sharding: pure data parallel over batch. Core k gets batches [2k, 2k+1]
(128 images of 512x512 per core), weight/bias replicated.

Per-core layout: partition p = (b_local*64 + c) -> one full image per
partition. Each iteration DMAs 16 input rows per partition (32 KiB
contiguous per-partition descriptors -> ~26.7 GB/s per SDMA engine vs
24.9 at 16 KiB; 4 MiB per dma_start), does the vertical pool with one
tensor_tensor add (row pairs are adjacent in the free dim), the
horizontal pool with a stride-2 tensor_tensor add, and the per-channel
affine on the scalar engine (scale/bias are per-partition [128,1]
scalars since partition == (b_local, c)).

The last chunk is split into 4 small pieces (tail_split) so the final
load->DVE->ACT->store drain chain is short, and the affine constant
load rides the gpsimd queue so the sync HWDGE ring's first descriptor
is the first big input load. Measured (quiet machine, core-0 NTFF
profile): 434 us with rows=8 defaults -> 407.5 us with this config;
DMA engines ~96% busy, 160 MiB/core at ~26.7 GB/s x 16 engines.
Run-to-run noise from neighbor tenants on shared HBM can inflate any
single measurement up to ~3x; compare via min over repeats.
"""

import numpy as np

import concourse.bacc as bacc
import concourse.bass as bass
import concourse.mybir as mybir
import concourse.tile as tile
from concourse.bass_utils import run_bass_kernel_spmd

N_CORES = 8
B, C, S = 16, 64, 512
B_LOC = B // N_CORES            # 2 batches per core
P = B_LOC * C                   # 128 partitions = one image per partition
IMG = S * S                     # 262144 input elems per image
OS = S // 2                     # 256
OUT_IMG = OS * OS               # 65536 output elems per image
ROWS_PER_ITER = 16              # input rows loaded per iteration (default)
CHUNK = ROWS_PER_ITER * S       # 8192 elems per partition per load (32 KiB)
N_ITERS = IMG // CHUNK          # 32 (last one split into 4 drain pieces)
OUT_CHUNK = CHUNK // 4          # 2048 elems per partition per store

FP32 = mybir.dt.float32

_nc_cache = None


def _build(reps=1, rows=16, ibufs=4, vbufs=2, hbufs=2, obufs=3,
           store_eng="scalar", split_load=1, loop_n=0,
           inplace_v=False, inplace_y=False, const_eng="gpsimd",
           tail_split=4, load_alt=False, store_group=1):
    # Bacc (not raw Bass): its finalize pass splits multi-sem waits into
    # event-semaphore instructions — TRN2 allows at most 1 wait per inst.
    # reps>1 repeats the full pass back-to-back in one NEFF (delta-timing).
    nc = bacc.Bacc("TRN2", target_bir_lowering=False, debug=False,
                   num_devices=N_CORES)
    chunk = rows * S             # input elems per partition per iteration
    n_iters = IMG // chunk
    out_chunk = chunk // 4

    x = nc.declare_dram_parameter("x", [P, IMG], FP32, isOutput=False)
    # affine[:, 0] = weight[c] / 4 (pool norm folded in), affine[:, 1] = bias[c]
    # (host-precomputed, already broadcast to the 128 partition images)
    affine = nc.declare_dram_parameter("affine", [P, 2], FP32, isOutput=False)
    out = nc.declare_dram_parameter("out", [P, OUT_IMG], FP32, isOutput=True)

    engs = {"sync": nc.sync, "scalar": nc.scalar, "gpsimd": nc.gpsimd}
    store = engs[store_eng]
    ceng = engs[const_eng]

    # work items: (elem offset, elems). tail_split>1 shortens the drain
    # chain by splitting the final chunk into smaller pieces.
    items = []
    full = n_iters - 1 if tail_split > 1 else n_iters
    items += [(i * chunk, chunk) for i in range(full)]
    if tail_split > 1:
        sub = chunk // tail_split
        assert (rows // tail_split) % 2 == 0 and rows % tail_split == 0
        base = full * chunk
        items += [(base + s * sub, sub) for s in range(tail_split)]

    # group consecutive full-size items so their outputs share one big
    # store DMA (store_group * chunk/4 elems per partition per store)
    groups = []
    cur = []
    for off, sz in items:
        if store_group > 1 and sz == chunk:
            cur.append((off, sz))
            if len(cur) == store_group:
                groups.append(cur)
                cur = []
        else:
            if cur:
                groups.append(cur)
                cur = []
            groups.append([(off, sz)])
    if cur:
        groups.append(cur)

    with tile.TileContext(nc) as tc:
        with tc.tile_pool(name="consts", bufs=1) as cpool, \
             tc.tile_pool(name="ld", bufs=ibufs) as ipool, \
             tc.tile_pool(name="vmid", bufs=vbufs) as vpool, \
             tc.tile_pool(name="hmid", bufs=hbufs) as hpool, \
             tc.tile_pool(name="st", bufs=obufs) as opool:

            cb = cpool.tile([P, 2], FP32)
            ceng.dma_start(out=cb[:], in_=affine[:, :])
            s_ap = cb[:, 0:1]
            b_ap = cb[:, 1:2]

            import contextlib
            loop_ctx = tc.For_i(0, loop_n, 1) if loop_n else \
                contextlib.nullcontext()
            with loop_ctx:
              it = 0
              for grp in groups * reps:
                g_off, g_sz = grp[0][0], sum(sz for _, sz in grp)
                y = opool.tile([P, g_sz // 4], FP32, name="y", tag="y")
                for gi, (off, sz) in enumerate(grp):
                    t = ipool.tile([P, sz], FP32, name="t", tag="t")
                    ldeng = (nc.sync, nc.scalar)[it % 2] if load_alt \
                        else nc.sync
                    it += 1
                    if split_load == 1:
                        ldeng.dma_start(out=t[:], in_=x[:, off:off + sz])
                    else:
                        part = sz // split_load
                        for s_ in range(split_load):
                            ldeng.dma_start(
                                out=t[:, s_ * part:(s_ + 1) * part],
                                in_=x[:, off + s_ * part:
                                      off + (s_ + 1) * part])

                    # vertical pool: rows 2r and 2r+1 sit at free-dim
                    # offsets (2r*S, (2r+1)*S) -> contiguous-stride add.
                    # Writing the result into the front of t is safe: the
                    # DVE streams monotonically and every write index
                    # trails its reads.
                    tv = t[:].rearrange("p (r two w) -> p r two w",
                                        two=2, w=S)
                    v_ap = t[:, 0:sz // 2] if inplace_v else \
                        vpool.tile([P, sz // 2], FP32, name="v", tag="v")[:]
                    vv = v_ap.rearrange("p (r w) -> p r w", w=S)
                    nc.vector.tensor_add(vv, tv[:, :, 0, :], tv[:, :, 1, :])

                    # horizontal pool: adjacent column pairs, stride-2
                    y_sl = y[:, gi * (sz // 4):(gi + 1) * (sz // 4)]
                    vh = v_ap.rearrange("p (r j two) -> p r j two",
                                        two=2, j=OS)
                    h_ap = y_sl if inplace_y else \
                        hpool.tile([P, sz // 4], FP32, name="h", tag="h")[:]
                    hh = h_ap.rearrange("p (r j) -> p r j", j=OS)
                    nc.vector.tensor_add(hh, vh[:, :, :, 0], vh[:, :, :, 1])

                    # per-channel affine on the scalar engine:
                    # y = Identity(h * (w[c]/4) + bias[c])
                    nc.scalar.activation(y_sl, h_ap,
                                         mybir.ActivationFunctionType.Identity,
                                         bias=b_ap, scale=s_ap)

                store.dma_start(out=out[:, g_off // 4:(g_off + g_sz) // 4],
                                in_=y[:])

    # run Bacc's legalization passes (multi-wait splitting, reg alloc, ...);
    # run_bass_via_pjrt serializes nc.m as-is and never finalizes.
    nc.finalize()
    return nc


def _get_nc():
    global _nc_cache
    if _nc_cache is None:
        _nc_cache = _build()
    return _nc_cache


def _make_in_maps(x, weight, bias):
    x = np.ascontiguousarray(np.asarray(x, dtype=np.float32))
    weight = np.asarray(weight, dtype=np.float32).reshape(C)
    bias = np.asarray(bias, dtype=np.float32).reshape(C)
    affine = np.stack([np.tile(weight * 0.25, B_LOC),
                       np.tile(bias, B_LOC)], axis=1)
    affine = np.ascontiguousarray(affine, dtype=np.float32)  # [P, 2]
    in_maps = []
    for k in range(N_CORES):
        shard = np.ascontiguousarray(
            x[k * B_LOC:(k + 1) * B_LOC].reshape(P, IMG))
        in_maps.append({"x": shard, "affine": affine})
    return in_maps


def run_sharded(x, weight, bias, trace=False, build_kw=None, **kw):
    """Run the SPMD kernel; returns (full_output, BassKernelResults)."""
    nc = _build(**build_kw) if build_kw else _get_nc()
    res = run_bass_kernel_spmd(nc, _make_in_maps(x, weight, bias),
                               core_ids=list(range(N_CORES)), trace=trace, **kw)
    outs = [res.results[k]["out"].reshape(B_LOC, C, OS, OS)
            for k in range(N_CORES)]
    return np.concatenate(outs, axis=0), res


def kernel(x, weight, bias):
    out, _ = run_sharded(x, weight, bias, trace=False)
    return out



# revision 19
# speedup vs baseline: 1.6596x; 1.1934x over previous
"""2x2 average pool + per-channel affine on 8 TRN2 NeuronCores.

Problem: x (16, 64, 512, 512) f32 -> out (16, 64, 256, 256) f32
  out[b,c,i,j] = weight[c] * mean(x[b,c,2i:2i+2,2j:2j+2]) + bias[c]

Sharding: pure data parallel over batch. Core k gets batches [2k, 2k+1]
(128 images of 512x512 per core), weight/bias replicated.

Per-core layout: partition p = (b_local*64 + c) -> one full image per
partition. Each iteration DMAs 16 input rows per partition (32 KiB
contiguous per-partition descriptors -> ~26.7 GB/s per SDMA engine vs
24.9 at 16 KiB; 4 MiB per dma_start), does the vertical pool with one
tensor_tensor add (row pairs are adjacent in the free dim), the
horizontal pool with a stride-2 tensor_tensor add, and the per-channel
affine on the scalar engine (scale/bias are per-partition [128,1]
scalars since partition == (b_local, c)).

The last chunk is split into 4 small pieces (tail_split) so the final
load->DVE->ACT->store drain chain is short, and the affine constant
load rides the gpsimd queue so the sync HWDGE ring's first descriptor
is the first big input load. Measured (quiet machine, core-0 NTFF
profile): 434 us with rows=8 defaults -> 407.5 us with this config;
DMA engines ~96% busy, 160 MiB/core at ~26.7 GB/s x 16 engines.
Run-to-run noise from neighbor tenants on shared HBM can inflate any
single measurement up to ~3x; compare via min over repeats.
"""

import numpy as np

import concourse.bacc as bacc
import concourse.bass as bass
import concourse.mybir as mybir
import concourse.tile as tile
from concourse.bass_utils import run_bass_kernel_spmd

N_CORES = 8
B, C, S = 16, 64, 512
B_LOC = B // N_CORES            # 2 batches per core
P = B_LOC * C                   # 128 partitions = one image per partition
IMG = S * S                     # 262144 input elems per image
OS = S // 2                     # 256
OUT_IMG = OS * OS               # 65536 output elems per image
ROWS_PER_ITER = 16              # input rows loaded per iteration (default)
CHUNK = ROWS_PER_ITER * S       # 8192 elems per partition per load (32 KiB)
N_ITERS = IMG // CHUNK          # 32 (last one split into 4 drain pieces)
OUT_CHUNK = CHUNK // 4          # 2048 elems per partition per store

FP32 = mybir.dt.float32

_nc_cache = None


def _work_items(rows, tail_split):
    """(elem offset, elems) per load. tail_split>1 shortens the drain
    chain by splitting the final chunk into smaller pieces."""
    chunk = rows * S
    n_iters = IMG // chunk
    items = []
    full = n_iters - 1 if tail_split > 1 else n_iters
    items += [(i * chunk, chunk) for i in range(full)]
    if tail_split > 1:
        sub = chunk // tail_split
        assert (rows // tail_split) % 2 == 0 and rows % tail_split == 0
        base = full * chunk
        items += [(base + s * sub, sub) for s in range(tail_split)]
    return items


def _build(reps=1, rows=16, ibufs=4, vbufs=2, hbufs=2, obufs=3,
           store_eng="scalar", split_load=1, loop_n=0,
           inplace_v=False, inplace_y=False, const_eng="gpsimd",
           tail_split=4, load_alt=False, store_group=1,
           in_layout="batch", load_spk=False):
    # Bacc (not raw Bass): its finalize pass splits multi-sem waits into
    # event-semaphore instructions — TRN2 allows at most 1 wait per inst.
    # reps>1 repeats the full pass back-to-back in one NEFF (delta-timing).
    nc = bacc.Bacc("TRN2", target_bir_lowering=False, debug=False,
                   num_devices=N_CORES)
    chunk = rows * S             # input elems per partition per iteration
    n_iters = IMG // chunk
    out_chunk = chunk // 4

    # in_layout="iter": host pre-packs x so each load item is one dense
    # HBM block ([P, sz] with partition stride sz) instead of 128 runs
    # scattered at 1 MiB stride — better HBM locality for the read stream.
    if in_layout == "iter":
        x = nc.declare_dram_parameter("x", [P * IMG], FP32, isOutput=False)
    else:
        x = nc.declare_dram_parameter("x", [P, IMG], FP32, isOutput=False)
    # affine[:, 0] = weight[c] / 4 (pool norm folded in), affine[:, 1] = bias[c]
    # (host-precomputed, already broadcast to the 128 partition images)
    affine = nc.declare_dram_parameter("affine", [P, 2], FP32, isOutput=False)
    out = nc.declare_dram_parameter("out", [P, OUT_IMG], FP32, isOutput=True)

    engs = {"sync": nc.sync, "scalar": nc.scalar, "gpsimd": nc.gpsimd}
    store = engs[store_eng]
    ceng = engs[const_eng]

    items = _work_items(rows, tail_split)
    # flat-stream base offset of each item for in_layout="iter"
    ibases = [0]
    for _, sz_ in items:
        ibases.append(ibases[-1] + P * sz_)

    # group consecutive full-size items so their outputs share one big
    # store DMA (store_group * chunk/4 elems per partition per store)
    groups = []
    cur = []
    for off, sz in items:
        if store_group > 1 and sz == chunk:
            cur.append((off, sz))
            if len(cur) == store_group:
                groups.append(cur)
                cur = []
        else:
            if cur:
                groups.append(cur)
                cur = []
            groups.append([(off, sz)])
    if cur:
        groups.append(cur)

    with tile.TileContext(nc) as tc:
        with tc.tile_pool(name="consts", bufs=1) as cpool, \
             tc.tile_pool(name="ld", bufs=ibufs) as ipool, \
             tc.tile_pool(name="vmid", bufs=vbufs) as vpool, \
             tc.tile_pool(name="hmid", bufs=hbufs) as hpool, \
             tc.tile_pool(name="st", bufs=obufs) as opool:

            cb = cpool.tile([P, 2], FP32)
            ceng.dma_start(out=cb[:], in_=affine[:, :])
            s_ap = cb[:, 0:1]
            b_ap = cb[:, 1:2]

            import contextlib
            loop_ctx = tc.For_i(0, loop_n, 1) if loop_n else \
                contextlib.nullcontext()
            with loop_ctx:
              it = 0
              for grp in groups * reps:
                g_off, g_sz = grp[0][0], sum(sz for _, sz in grp)
                y = opool.tile([P, g_sz // 4], FP32, name="y", tag="y")
                for gi, (off, sz) in enumerate(grp):
                    t = ipool.tile([P, sz], FP32, name="t", tag="t")
                    ldeng = (nc.sync, nc.scalar)[it % 2] if load_alt \
                        else nc.sync
                    if in_layout == "iter":
                        fb = ibases[it % len(items)]
                        src = x[fb:fb + P * sz].rearrange(
                            "(p s) -> p s", p=P)
                    else:
                        src = x[:, off:off + sz]
                    it += 1
                    if split_load == 1:
                        ldeng.dma_start(out=t[:], in_=src,
                                        single_packet=load_spk)
                    else:
                        part = sz // split_load
                        for s_ in range(split_load):
                            ldeng.dma_start(
                                out=t[:, s_ * part:(s_ + 1) * part],
                                in_=src[:, s_ * part:(s_ + 1) * part])

                    # vertical pool: rows 2r and 2r+1 sit at free-dim
                    # offsets (2r*S, (2r+1)*S) -> contiguous-stride add.
                    # Writing the result into the front of t is safe: the
                    # DVE streams monotonically and every write index
                    # trails its reads.
                    tv = t[:].rearrange("p (r two w) -> p r two w",
                                        two=2, w=S)
                    v_ap = t[:, 0:sz // 2] if inplace_v else \
                        vpool.tile([P, sz // 2], FP32, name="v", tag="v")[:]
                    vv = v_ap.rearrange("p (r w) -> p r w", w=S)
                    nc.vector.tensor_add(vv, tv[:, :, 0, :], tv[:, :, 1, :])

                    # horizontal pool: adjacent column pairs, stride-2
                    y_sl = y[:, gi * (sz // 4):(gi + 1) * (sz // 4)]
                    vh = v_ap.rearrange("p (r j two) -> p r j two",
                                        two=2, j=OS)
                    h_ap = y_sl if inplace_y else \
                        hpool.tile([P, sz // 4], FP32, name="h", tag="h")[:]
                    hh = h_ap.rearrange("p (r j) -> p r j", j=OS)
                    nc.vector.tensor_add(hh, vh[:, :, :, 0], vh[:, :, :, 1])

                    # per-channel affine on the scalar engine:
                    # y = Identity(h * (w[c]/4) + bias[c])
                    nc.scalar.activation(y_sl, h_ap,
                                         mybir.ActivationFunctionType.Identity,
                                         bias=b_ap, scale=s_ap)

                store.dma_start(out=out[:, g_off // 4:(g_off + g_sz) // 4],
                                in_=y[:])

    # run Bacc's legalization passes (multi-wait splitting, reg alloc, ...);
    # run_bass_via_pjrt serializes nc.m as-is and never finalizes.
    nc.finalize()
    return nc


def _get_nc():
    global _nc_cache
    if _nc_cache is None:
        _nc_cache = _build()
    return _nc_cache


def _make_in_maps(x, weight, bias, in_layout="batch", rows=16,
                  tail_split=4):
    x = np.ascontiguousarray(np.asarray(x, dtype=np.float32))
    weight = np.asarray(weight, dtype=np.float32).reshape(C)
    bias = np.asarray(bias, dtype=np.float32).reshape(C)
    affine = np.stack([np.tile(weight * 0.25, B_LOC),
                       np.tile(bias, B_LOC)], axis=1)
    affine = np.ascontiguousarray(affine, dtype=np.float32)  # [P, 2]
    items = _work_items(rows, tail_split) if in_layout == "iter" else None
    in_maps = []
    for k in range(N_CORES):
        shard = np.ascontiguousarray(
            x[k * B_LOC:(k + 1) * B_LOC].reshape(P, IMG))
        if in_layout == "iter":
            shard = np.concatenate(
                [shard[:, off:off + sz].reshape(-1) for off, sz in items])
        in_maps.append({"x": shard, "affine": affine})
    return in_maps


def run_sharded(x, weight, bias, trace=False, build_kw=None, **kw):
    """Run the SPMD kernel; returns (full_output, BassKernelResults)."""
    bkw = build_kw or {}
    nc = _build(**build_kw) if build_kw else _get_nc()
    in_maps = _make_in_maps(
        x, weight, bias,
        in_layout=bkw.get("in_layout", "batch"),
        rows=bkw.get("rows", 16),
        tail_split=bkw.get("tail_split", 4))
    res = run_bass_kernel_spmd(nc, in_maps,
                               core_ids=list(range(N_CORES)), trace=trace, **kw)
    outs = [res.results[k]["out"].reshape(B_LOC, C, OS, OS)
            for k in range(N_CORES)]
    return np.concatenate(outs, axis=0), res


def kernel(x, weight, bias):
    out, _ = run_sharded(x, weight, bias, trace=False)
    return out

